# revision 1
# baseline (speedup 1.0000x reference)
"""AFNO2D block-MLP spectral layer on 8 TRN2 NeuronCores.

Math (per batch element, row r in [0,4096), channels C=768):
    y   = x @ cas                      (cas = Hartley matrix over channels)
    h_ri= relu(y_blk @ w1[ri] + b1)    (block-diagonal, 8 blocks of 96)
    o2r = h_r @ w2r - h_i @ w2i + b2r
    o2i = h_i @ w2r + h_r @ w2i + b2i
    d   = softshrink(o2r) - softshrink(o2i)     (lambda = 0.01)
    out = (d @ cas) / (B*N*C) + x

Sharding: data-parallel over batch B=8 -> one batch element per core.
No collectives needed; full inputs are sharded / outputs gathered on host.

Device layout: channels on partitions, rows on the free dim, processed in
8 chunks of 512 rows, fully pipelined (DMA in -> PE -> ACT/DVE/Pool
epilogues -> DMA out).

Key design points:
- All matmuls run in fp8e4 with MatmulPerfMode.DoubleRow (PE: 0.5
  cycles/row, contraction pairs of 128 partitions). Since the reference
  output is x + ~1e-6 * transform(x), fp8 noise in the transform path is
  ~1e-8 of the output; measured overall rel err ~1e-7.
- Host pre-processing: x is supplied twice (f32 for the residual add and
  pre-transposed fp8 for the matmul path); stage 1 (x @ cas) is fused
  into layer 1 by pre-multiplying cas into w1 (W1F = cas_blk @ w1, x4
  scale); w2 is pre-scaled x16 to sit in fp8's normal range (the 1/16
  folds into the softshrink threshold and the final output scale); b2 is
  injected on the PE via a rank-1 matmul against a constant ones tile so
  the o2 PSUM evacuation is a single bias-free paired copy.
- softshrink difference uses the identity ss(v) = v - clip(v, +-m):
  d = (va - vb) - clip(va) + clip(vb), with the clip pair done in one
  4x-mode tensor_scalar op on bf16.
- The 96-channel blocks are not 128-partition aligned; engine access
  patterns only allow partition bases 0/32/64, so d is assembled into the
  128-aligned layout with small SBUF->SBUF DMAs.
- All DRAM I/O uses chunk-major host layouts (x, x-transposed-fp8, and
  the output are rearranged on the host) so every per-chunk DMA is one
  contiguous run per partition -- 128 descriptors instead of 512-768.
- Engine balance (per the TimelineSim cost model): PE ~87us, ACT ~121us,
  DVE ~122us, Pool ~72us, SP(DMA issue) ~94us. Measured steady-state on
  hardware (8 cores in parallel): best observed 77us/iteration under an
  unloaded terminal -- at the ~80us DMA roofline (31 MB/core) -- with
  ~115-180us typical and ~200-250us medians when the shared terminal is
  loaded.
"""

import numpy as np


B, N, C = 8, 4096, 768
NB, BS = 8, 96          # blocks, block size
NT = C // 128           # 6 channel tiles of 128 partitions
CHUNK = 512             # rows per pipeline chunk
NCHUNK = N // CHUNK
NRT = CHUNK // 128
LAM = 0.01
INV_N = 1.0 / float(B * N * C)
SCALE = 16.0  # fp8 dynamic-range scale for the MLP weights

_CACHE = {}

# tuning knobs: pool buffer counts and psum tag assignment
CFG = {
    "big_bufs": 3,
    "mid_bufs": 3,
    "tmp_bufs": 4,
    # psum 2-bank tiles per tag; 2*(psh+psab+pso) <= 8
    "psh": 2, "psab": 2, "pso": 1,
    "bias_mm": True,   # inject b2 via ones-matmul (PE) vs bias-add on evac
    "out_split": False,  # final epilogue: ACT scale-copy + Pool add vs DVE stt
    "vab_act": 8,        # of 8 vab evacuations per chunk, how many on ACT
    "dt_gpsimd": False,  # dT assembly slivers on gpsimd ring vs SP ring
    "b1_mm": False,    # inject b1 via ones-matmul; h evac splits ACT/DVE
    "h_dve": 6,        # of 16 h-evacs per chunk, how many go to DVE
}


# Legal engine partition windows: base 0 (len<=128), base 32 (len<=32),
# base 64 (len<=64).  Blocks of 96 land at tile offsets {0, 96, 64, 32}.

def _l1_panels():
    """Layer-1 weight panels: list of (block k, channel tile t).

    Every matmul is a full K=128 read of yT tile t; weight rows outside
    block k are zero (mixing sub-128 partition bases inside one PSUM
    accumulation group crashes the PE)."""
    panels = []
    for k in range(NB):
        ch0 = BS * k
        for t in range(ch0 // 128, (ch0 + BS - 1) // 128 + 1):
            panels.append((k, t))
    return panels


def _d_pieces(k):
    """d-write pieces for block k: (tile, base, length, src_off, via_dma)."""
    ch0 = BS * k
    t0, b = ch0 // 128, ch0 % 128
    if b == 0:
        rel = [(0, 0, 96, 0, False)]
    elif b == 32:
        rel = [(0, 32, 32, 0, False), (0, 64, 32, 32, False),
               (0, 96, 32, 64, True)]
    elif b == 64:
        rel = [(0, 64, 64, 0, False), (1, 0, 32, 64, False)]
    else:  # b == 96
        rel = [(0, 96, 32, 0, True), (1, 0, 32, 32, False),
               (1, 32, 32, 64, False)]
    return [(t0 + tr, base, ln, so, dma) for (tr, base, ln, so, dma) in rel]


def _build(repeat=1, compile=True, cfg=None):
    from contextlib import ExitStack
    import concourse.tile as tile
    from concourse import bacc, mybir

    f32 = mybir.dt.float32
    bf16 = mybir.dt.bfloat16
    f8 = mybir.dt.float8e4
    DR = mybir.MatmulPerfMode.DoubleRow
    AF = mybir.ActivationFunctionType
    ALU = mybir.AluOpType
    M = SCALE * LAM  # softshrink threshold in the scaled domain
    S1 = 4.0        # fp8 scale for the fused cas@w1 weights

    cfg = dict(CFG, **(cfg or {}))
    nc = bacc.Bacc("TRN2", target_bir_lowering=False, debug=False, num_devices=8)
    x_ap = nc.dram_tensor("xh", [128, NCHUNK, NRT, C], f32, kind="ExternalInput").ap()
    xt8_ap = nc.dram_tensor("xt8h", [128, NCHUNK, NT, CHUNK], f8, kind="ExternalInput").ap()
    cas_ap = nc.dram_tensor("cas8", [128, NT // 2, 2, C], f8, kind="ExternalInput").ap()
    w1_ap = nc.dram_tensor("w1f", [128, NT // 2, 2, 2, NB, BS], f8, kind="ExternalInput").ap()
    w2_ap = nc.dram_tensor("w2dr", [BS, 2, 2, NB, BS], f8, kind="ExternalInput").ap()
    b1_ap = nc.dram_tensor("b1s", [BS, 2 * NB], f32, kind="ExternalInput").ap()
    b2_ap = nc.dram_tensor("b2s", [BS, 2 * NB], f32, kind="ExternalInput").ap()
    w2b_ap = nc.dram_tensor("w2b", [128, 2, 2, NB, BS], f8, kind="ExternalInput").ap()
    w1bb_ap = nc.dram_tensor("w1bb", [128, 2, 2, NB, BS], f8, kind="ExternalInput").ap()
    out_ap = nc.dram_tensor("out", [128, NCHUNK, NRT, C], f32, kind="ExternalOutput").ap()

    with tile.TileContext(nc) as tc, ExitStack() as ctx:
        consts = ctx.enter_context(tc.tile_pool(name="consts", bufs=1))
        sb = ctx.enter_context(tc.tile_pool(name="sb", bufs=cfg["big_bufs"]))
        mid = ctx.enter_context(tc.tile_pool(name="mid", bufs=cfg["mid_bufs"]))
        tmp = ctx.enter_context(tc.tile_pool(name="tmp", bufs=cfg["tmp_bufs"]))
        pools = {}
        for tag in ("psh", "psab", "pso"):
            pools[tag] = ctx.enter_context(
                tc.tile_pool(name=tag, bufs=cfg[tag], space="PSUM"))

        cas_sb = consts.tile([128, NT // 2, 2, C], f8)
        nc.sync.dma_start(out=cas_sb[:], in_=cas_ap[:])
        w1_sb = consts.tile([128, NT // 2, 2, 2, NB, BS], f8)
        nc.sync.dma_start(out=w1_sb[:], in_=w1_ap[:])
        w2_sb = consts.tile([BS, 2, 2, NB, BS], f8)
        nc.sync.dma_start(out=w2_sb[:], in_=w2_ap[:])
        if not cfg["b1_mm"]:
            b1_sb = consts.tile([BS, 2 * NB], f32)
            nc.sync.dma_start(out=b1_sb[:], in_=b1_ap[:])
        if not cfg["bias_mm"]:
            b2_sb = consts.tile([BS, 2 * NB], f32)
            nc.sync.dma_start(out=b2_sb[:], in_=b2_ap[:])
        if cfg["bias_mm"]:
            w2b_sb = consts.tile([128, 2, 2, NB, BS], f8)
            nc.vector.memset(w2b_sb[:], 0.0)
            nc.sync.dma_start(out=w2b_sb[0:1], in_=w2b_ap[0:1])
        if cfg["b1_mm"]:
            w1bb_sb = consts.tile([128, 2, 2, NB, BS], f8)
            nc.vector.memset(w1bb_sb[:], 0.0)
            nc.sync.dma_start(out=w1bb_sb[0:1], in_=w1bb_ap[0:1])
        ones8 = consts.tile([128, 2, CHUNK], f8)
        nc.vector.memset(ones8[:], 1.0)

        for rep in range(repeat):
          for c in range(NCHUNK):
            r0 = c * CHUNK
            # ---- load x chunk (residual) and pre-transposed fp8 x
            xt = sb.tile([128, NRT, C], f32, tag="xt")
            nc.sync.dma_start(out=xt[:], in_=x_ap[:, c, :, :])
            xT8 = mid.tile([128, NT, CHUNK], f8, tag="xT8")
            nc.sync.dma_start(out=xT8[:], in_=xt8_ap[:, c, :, :])

            # ---- fused stage1+layer1 (DoubleRow fp8):
            #      h = relu((x @ (cas_blk@w1) * S1) / S1 + b1), stored fp8
            h_sb = sb.tile([BS, 2 * NB, CHUNK], f8, tag="h")
            for k in range(NB):
                for ri in range(2):
                    psh = pools["psh"].tile([BS, CHUNK], f32, tag="psh")
                    for j in range(NT // 2):
                        nc.tensor.matmul(
                            psh[:],
                            w1_sb[:, j, :, ri, k, :],
                            xT8[:, 2 * j:2 * j + 2, :],
                            start=(j == 0),
                            stop=(j == NT // 2 - 1) and not cfg["b1_mm"],
                            perf_mode=DR,
                        )
                    if cfg["b1_mm"]:
                        nc.tensor.matmul(
                            psh[:], w1bb_sb[:, :, ri, k, :], ones8[:],
                            start=False, stop=True, perf_mode=DR,
                        )
                        if (k * 2 + ri) % 16 < cfg["h_dve"]:
                            nc.vector.tensor_scalar(
                                h_sb[:, k * 2 + ri, :], psh[:],
                                1.0 / S1, 0.0, ALU.mult, ALU.max)
                        else:
                            nc.scalar.activation(
                                h_sb[:, k * 2 + ri, :], psh[:], AF.Relu,
                                scale=1.0 / S1,
                            )
                    else:
                        nc.scalar.activation(
                            h_sb[:, k * 2 + ri, :], psh[:], AF.Relu,
                            bias=b1_sb[:, k * 2 + ri:k * 2 + ri + 1],
                            scale=1.0 / S1,
                        )

            # ---- layer 2 (DoubleRow fp8, x16-scaled, b2 via ones-matmul)
            # psab = 16*o2{r,i}; with m = 16*lam:
            #   d*16 = (va - vb) - clip(va, +-m) + clip(vb, +-m)
            dT = mid.tile([128, NT, CHUNK], f8, tag="dT")
            for k in range(NB):
                psab = pools["psab"].tile([BS, 2, CHUNK], f32, tag="psab")
                hp = h_sb[:, 2 * k:2 * k + 2, :]
                vab = tmp.tile([BS, 2, CHUNK], bf16, tag="vab")
                if cfg["bias_mm"]:
                    for j in range(2):
                        nc.tensor.matmul(psab[:, j, :], w2_sb[:, :, j, k, :],
                                         hp, start=True, stop=False,
                                         perf_mode=DR)
                        nc.tensor.matmul(psab[:, j, :], w2b_sb[:, :, j, k, :],
                                         ones8[:], start=False, stop=True,
                                         perf_mode=DR)
                    if k % 8 < cfg["vab_act"]:
                        nc.scalar.activation(vab[:], psab[:], AF.Copy)
                    else:
                        nc.vector.tensor_copy(out=vab[:], in_=psab[:])
                else:
                    for j in range(2):
                        nc.tensor.matmul(psab[:, j, :], w2_sb[:, :, j, k, :],
                                         hp, start=True, stop=True,
                                         perf_mode=DR)
                        bcol = b2_sb[:, k * 2 + j:k * 2 + j + 1]
                        if (2 * k + j) % 16 < 10:
                            nc.scalar.add(vab[:, j, :], psab[:, j, :], bcol)
                        else:
                            nc.vector.tensor_scalar_add(
                                vab[:, j, :], psab[:, j, :], bcol)
                va, vb = vab[:, 0, :], vab[:, 1, :]
                q = tmp.tile([BS, CHUNK], bf16, tag="q")
                cab = tmp.tile([BS, 2, CHUNK], bf16, tag="cab")
                r = tmp.tile([BS, CHUNK], bf16, tag="r")
                nc.vector.tensor_sub(q[:], va, vb)
                nc.vector.tensor_scalar(cab[:], vab[:], -M, M, ALU.max, ALU.min)
                nc.vector.tensor_sub(r[:], q[:], cab[:, 0, :])
                dblk = tmp.tile([BS, CHUNK], f8, tag="dblk")
                nc.gpsimd.tensor_add(dblk[:], r[:], cab[:, 1, :])
                ch0 = BS * k
                t0, bb = ch0 // 128, ch0 % 128
                eng = nc.gpsimd if cfg["dt_gpsimd"] else nc.sync
                if bb + BS <= 128:
                    eng.dma_start(
                        out=dT[bb:bb + BS, t0, :], in_=dblk[:])
                else:
                    l0 = 128 - bb
                    eng.dma_start(
                        out=dT[bb:128, t0, :], in_=dblk[0:l0, :])
                    eng.dma_start(
                        out=dT[0:BS - l0, t0 + 1, :], in_=dblk[l0:BS, :])

            # ---- final (DoubleRow fp8): out = (dT.T @ cas) * inv_n/64 + x
            out_sb = sb.tile([128, NRT, C], f32, tag="out_sb")
            for rt in range(NRT):
                pso = pools["pso"].tile([128, 2, 512], f32, tag="pso")
                for half in range(2):
                    for j in range(NT // 2):
                        nc.tensor.matmul(
                            pso[:, half, 0:384],
                            dT[:, 2 * j:2 * j + 2, rt * 128:(rt + 1) * 128],
                            cas_sb[:, j, :, half * 384:half * 384 + 384],
                            start=(j == 0),
                            stop=(j == NT // 2 - 1),
                            perf_mode=DR,
                        )
                if cfg["out_split"]:
                    os_t = tmp.tile([128, 2, 384], f32, tag="os")
                    nc.scalar.activation(
                        os_t[:], pso[:, :, 0:384], AF.Copy,
                        scale=INV_N / SCALE)
                    nc.gpsimd.tensor_add(
                        out_sb[:, rt, :].rearrange("p (h w) -> p h w", h=2),
                        os_t[:],
                        xt[:, rt, :].rearrange("p (h w) -> p h w", h=2))
                else:
                    nc.vector.scalar_tensor_tensor(
                        out=out_sb[:, rt, :].rearrange("p (h w) -> p h w", h=2),
                        in0=pso[:, :, 0:384],
                        scalar=INV_N / SCALE,
                        in1=xt[:, rt, :].rearrange("p (h w) -> p h w", h=2),
                        op0=ALU.mult, op1=ALU.add)
            nc.sync.dma_start(out=out_ap[:, c, :, :], in_=out_sb[:])

    if compile:
        nc.compile()
    return nc


def _prep_inputs(x, w1, b1, w2, b2):
    import ml_dtypes
    f8np = ml_dtypes.float8_e4m3

    n = np.arange(C, dtype=np.float64)
    ang = 2.0 * np.pi * n[:, None] * n[None, :] / C
    cas = (np.cos(ang) + np.sin(ang)).astype(np.float32)
    # cas8[p, j, s, c] = cas[(2j+s)*128 + p, c]
    cas8 = np.ascontiguousarray(
        cas.reshape(NT // 2, 2, 128, C).transpose(2, 0, 1, 3)).astype(f8np)

    # fused stage1+layer1 weights: W1F[ci, ri, k, m] = sum_c cas[ci, 96k+c]
    # * w1[ri, k, c, m], scaled by S1=4 for fp8 range, in DoubleRow layout
    cas_blocks = cas.reshape(C, NB, BS)
    w1f_full = 4.0 * np.einsum(
        'akb,rkbm->arkm', cas_blocks.astype(np.float64),
        w1.astype(np.float64)).astype(np.float32)       # [768, 2, NB, 96]
    w1f = np.ascontiguousarray(
        w1f_full.reshape(NT // 2, 2, 128, 2, NB, BS).transpose(2, 0, 1, 3, 4, 5)
    ).astype(f8np)

    # layer-2 weights (xSCALE): pair dim = (h_r, h_i); out pair j: 0->o2r, 1->o2i
    w2dr = np.zeros((BS, 2, 2, NB, BS), np.float32)
    w2dr[:, 0, 0] = SCALE * w2[0].transpose(1, 0, 2)   # w2r applied to h_r
    w2dr[:, 1, 0] = -SCALE * w2[1].transpose(1, 0, 2)  # -w2i applied to h_i
    w2dr[:, 0, 1] = SCALE * w2[1].transpose(1, 0, 2)   # w2i applied to h_r
    w2dr[:, 1, 1] = SCALE * w2[0].transpose(1, 0, 2)   # w2r applied to h_i
    w2dr = w2dr.astype(f8np)

    b1s = np.ascontiguousarray(
        b1.transpose(2, 1, 0).reshape(BS, 2 * NB)).astype(np.float32)
    # b2 as rank-1 matmul weights against a constant ones tile
    w2b = np.zeros((128, 2, 2, NB, BS), np.float32)
    w2b[0, 0] = SCALE * b2
    w2b = w2b.astype(f8np)

    w1bb = np.zeros((128, 2, 2, NB, BS), np.float32)
    w1bb[0, 0] = 4.0 * b1   # S1 * b1 at partition 0, pair-slot 0
    w1bb = w1bb.astype(f8np)
    b2s = np.ascontiguousarray(
        (SCALE * b2).transpose(2, 1, 0).reshape(BS, 2 * NB)).astype(np.float32)
    shared = {"cas8": cas8, "w1f": w1f, "w2dr": w2dr,
              "b1s": b1s, "b2s": b2s, "w2b": w2b, "w1bb": w1bb}
    maps = []
    for i in range(B):
        xi = np.asarray(x[i], dtype=np.float32)
        # xh[p, c, s, ch] = x[c*512 + s*128 + p, ch]
        xh = np.ascontiguousarray(
            xi.reshape(NCHUNK, NRT, 128, C).transpose(2, 0, 1, 3))
        # xt8h[p, c, t, r] = x[c*512 + r, t*128 + p]
        xt8h = np.ascontiguousarray(
            xi.T.astype(f8np).reshape(NT, 128, NCHUNK, CHUNK)
            .transpose(1, 2, 0, 3))
        maps.append({"xh": xh, "xt8h": xt8h, **shared})
    return maps


class _Runner:
    """Persistent jitted shard_map runner for a compiled Bass module.

    Mirrors bass2jax.run_bass_via_pjrt's multi-core path but keeps the
    jitted callable (and hence the compiled NEFF) alive across calls."""

    def __init__(self, nc):
        import jax
        from jax.sharding import Mesh, PartitionSpec, NamedSharding
        from jax.experimental.shard_map import shard_map
        from concourse import mybir
        from concourse.bass2jax import (
            _bass_exec_p, install_neuronx_cc_hook, partition_id_tensor)

        install_neuronx_cc_hook()
        self.jax = jax
        self.nc = nc
        pid_name = nc.partition_id_tensor.name if nc.partition_id_tensor else None
        in_names, out_names, out_avals = [], [], []
        for alloc in nc.m.functions[0].allocations:
            if not isinstance(alloc, mybir.MemoryLocationSet):
                continue
            name = alloc.memorylocations[0].name
            if alloc.kind == "ExternalInput":
                if name != pid_name:
                    in_names.append(name)
            elif alloc.kind == "ExternalOutput":
                out_names.append(name)
                out_avals.append(jax.core.ShapedArray(
                    tuple(alloc.tensor_shape), mybir.dt.np(alloc.dtype)))
        self.in_names, self.out_names, self.out_avals = in_names, out_names, out_avals

        def _body(*args):
            operands = list(args)
            if pid_name is not None:
                operands.append(partition_id_tensor())
            all_names = tuple(in_names) + tuple(out_names) + (
                (pid_name,) if pid_name else ())
            outs = _bass_exec_p.bind(
                *operands,
                out_avals=tuple(out_avals),
                in_names=all_names,
                out_names=tuple(out_names),
                lowering_input_output_aliases=(),
                sim_require_finite=True,
                sim_require_nnan=True,
                nc=nc,
            )
            return tuple(outs)

        devices = jax.devices()[:B]
        self.mesh = Mesh(np.asarray(devices), ("core",))
        nargs = len(in_names) + len(out_names)
        self.sharding = NamedSharding(self.mesh, PartitionSpec("core"))
        self.f = jax.jit(shard_map(
            _body, mesh=self.mesh,
            in_specs=(PartitionSpec("core"),) * nargs,
            out_specs=(PartitionSpec("core"),) * len(out_names),
            check_rep=False,
        ))

    def device_args(self, in_maps):
        concat = [
            np.concatenate([np.asarray(m[n]) for m in in_maps], axis=0)
            for n in self.in_names
        ]
        concat += [
            np.zeros((len(in_maps) * a.shape[0], *a.shape[1:]), a.dtype)
            for a in self.out_avals
        ]
        return [self.jax.device_put(a, self.sharding) for a in concat]

    def run(self, in_maps):
        outs = self.f(*self.device_args(in_maps))
        n = len(in_maps)
        return [
            np.asarray(outs[i]).reshape(n, *self.out_avals[i].shape)
            for i in range(len(self.out_names))
        ]


def get_runner(repeat=1):
    key = ("runner", repeat)
    if key not in _CACHE:
        _CACHE[key] = _Runner(_build(repeat=repeat))
    return _CACHE[key]


def kernel(x, w1, b1, w2, b2):
    x = np.asarray(x, dtype=np.float32)
    w1 = np.asarray(w1, dtype=np.float32)
    b1 = np.asarray(b1, dtype=np.float32)
    w2 = np.asarray(w2, dtype=np.float32)
    b2 = np.asarray(b2, dtype=np.float32)
    runner = get_runner(1)
    in_maps = _prep_inputs(x, w1, b1, w2, b2)
    outh = runner.run(in_maps)[0]          # [B, 128, NCHUNK, NRT, C]
    out = outh.transpose(0, 2, 3, 1, 4).reshape(B, N, C)
    return np.ascontiguousarray(out).astype(np.float32)



# revision 2
# speedup vs baseline: 2.7331x; 2.7331x over previous
"""AFNO2D block-MLP spectral layer on 8 TRN2 NeuronCores — v2.

Math per batch element (rows r in [0,4096), channels C=768):
    y   = x @ cas                     (cas = Hartley matrix over channels)
    h   = relu(y_blk @ w1[ri] + b1)   (block-diagonal, 8 blocks of 96)
    o2r = h_r @ w2r - h_i @ w2i + b2r ; o2i = h_i @ w2r + h_r @ w2i + b2i
    d   = softshrink(o2r) - softshrink(o2i)       (lambda = 0.01)
    out = (d @ cas) / (B*N*C) + x

Sharding: data-parallel over batch B=8 (one element/core, no collectives).

v2 design vs v1:
- 128-dense channel packing: h-space (1536 ch) packed as 12 tiles of 128
  partitions; o2/q/d-space (768 ch) as 6 tiles. L1 = 36 matmuls/chunk
  (vs 48), L2 = 24 zero-padded 2-tile-window DR matmuls + 12 rank-1 bias
  matmuls, final = 24 (3 dense DR passes). No sliver DMAs (d tiles are
  written whole at partition base 0).
- Softshrink chain reads PSUM directly (no bf16 staging copy):
  q = va - vb, cab = clip(va,vb) in one 2-slice op, r = q - cab_r (bf16
  2x mode), d = r + cab_i -> fp8.
- Residual add moved to host: device reads only xT (fp8, 3.1MB) and
  writes the bf16 delta (6.3MB); final evac is a plain ACT scale-copy.
- Per-op engine assignment (ACT/DVE/GPSIMD) is a tunable config.
"""

import numpy as np


B, N, C = 8, 4096, 768
NB, BS = 8, 96
NT = C // 128            # 6 o2/d channel tiles
NHT = 2 * NB * BS // 128  # 12 h channel tiles
CHUNK = 512
NCHUNK = N // CHUNK
NRT = CHUNK // 128
LAM = 0.01
INV_N = 1.0 / float(B * N * C)
SCALE = 16.0             # psab domain = SCALE * o2
S1 = 4.0                 # h' = S1 * h_true ; w2p = (SCALE/S1) * w2
M = SCALE * LAM          # softshrink threshold in psab domain

_CACHE = {}

# L2 window table: o2-tile tt needs h-chans [192*k0, 192*k0+384)
def _l2_windows(tt):
    k0 = (128 * tt) // 96
    a = (192 * k0) // 128
    b = (192 * k0 + 383) // 128
    return [(a, (a, a + 1)), (b - 1, tuple(range(a + 2, b + 1)))]


# engine assignment per op class (tunable): 'a'=ACT, 'd'=DVE, 'g'=GPSIMD
# Legality: GPSIMD cannot touch PSUM; tensor_tensor allows at most one PSUM
# operand. So psum-consuming ops (A, VB, F) are ACT/DVE only; the ss chain
# (C, Q, R, E) runs on SBUF bf16 data and may use GPSIMD.
CFG = {
    "a_eng": "aaaaaaaaaaaa",   # 12 L1 evacs relu+bias psum->fp8 (a/d)
    "vb_eng": "aadaadaadaad",  # 12 biased evacs psab_j -> vab_j bf16 (a/d)
    "c_eng": "dddddd",         # 6 clip pairs [128,2,512] vab->cab bf16 4x (d/g)
    "q_eng": "dddddd",         # 6 q = vab0 - vab1 bf16 2x (d/g)
    "r_eng": "dddddd",         # 6 r = q - cab0 bf16 2x (d/g)
    "e_eng": "gggggg",         # 6 d = r + cab1 -> fp8 (d/g)
    "f_eng": "addddddd",       # 8 final evacs [128,384] psum->bf16 (a/d)
    "f_dma": False,            # PSUM->DRAM DMA unsupported (SBUF/DRAM only)
    "lag": True,               # emit (C,Q,R,E) one tt behind (VB pipelining)
    "final_lag": 3,            # emit final stage N chunks behind its d
    "stages": "full",          # debug: l1 / l2 / ss / full
    "big_bufs": 3,
    "mid_bufs": 4,
    "tmp_bufs": 4,
    "psh": 2, "psab": 2, "pso": 2,
}


def _build(repeat=1, compile=True, cfg=None):
    from contextlib import ExitStack
    import concourse.tile as tile
    from concourse import bacc, mybir

    f32 = mybir.dt.float32
    bf16 = mybir.dt.bfloat16
    f8 = mybir.dt.float8e4
    DR = mybir.MatmulPerfMode.DoubleRow
    AF = mybir.ActivationFunctionType
    ALU = mybir.AluOpType

    cfg = dict(CFG, **(cfg or {}))
    nc = bacc.Bacc("TRN2", target_bir_lowering=False, debug=False, num_devices=8)
    xt8_ap = nc.dram_tensor("xt8h", [128, NCHUNK, NT, CHUNK], f8, kind="ExternalInput").ap()
    cas_ap = nc.dram_tensor("cas8", [128, NT // 2, 2, C], f8, kind="ExternalInput").ap()
    w1_ap = nc.dram_tensor("w1p", [128, 3, 2, NHT, 128], f8, kind="ExternalInput").ap()
    w2_ap = nc.dram_tensor("w2p", [128, 2, 2, NT, 2, 128], f8, kind="ExternalInput").ap()
    b1_ap = nc.dram_tensor("b1p", [128, NHT], f32, kind="ExternalInput").ap()
    b2_ap = nc.dram_tensor("b2p", [128, 2, NT], f32, kind="ExternalInput").ap()
    out_dt = f32 if cfg["f_dma"] else bf16
    out_ap = nc.dram_tensor("out", [128, NCHUNK, NRT, 2, 384], out_dt, kind="ExternalOutput").ap()

    def eng(ch):
        return {"a": nc.scalar, "d": nc.vector, "g": nc.gpsimd}[ch]

    with tile.TileContext(nc) as tc, ExitStack() as ctx:
        consts = ctx.enter_context(tc.tile_pool(name="consts", bufs=1))
        sb = ctx.enter_context(tc.tile_pool(name="sb", bufs=cfg["big_bufs"]))
        mid = ctx.enter_context(tc.tile_pool(name="mid", bufs=cfg["mid_bufs"]))
        tmp = ctx.enter_context(tc.tile_pool(name="tmp", bufs=cfg["tmp_bufs"]))
        pools = {}
        for tag in ("psh", "psab", "pso"):
            pools[tag] = ctx.enter_context(
                tc.tile_pool(name=tag, bufs=cfg[tag], space="PSUM"))

        cas_sb = consts.tile([128, NT // 2, 2, C], f8)
        nc.sync.dma_start(out=cas_sb[:], in_=cas_ap[:])
        w1_sb = consts.tile([128, 3, 2, NHT, 128], f8)
        nc.sync.dma_start(out=w1_sb[:], in_=w1_ap[:])
        w2_sb = consts.tile([128, 2, 2, NT, 2, 128], f8)
        nc.sync.dma_start(out=w2_sb[:], in_=w2_ap[:])
        b1_sb = consts.tile([128, NHT], f32)
        nc.sync.dma_start(out=b1_sb[:], in_=b1_ap[:])
        b2_sb = consts.tile([128, 2, NT], f32)
        nc.sync.dma_start(out=b2_sb[:], in_=b2_ap[:])

        def emit_final(c, d_sb):
            # delta = (d @ cas); evac scaled to bf16 (or raw f32 DMA)
            out_sb = None
            if not cfg["f_dma"]:
                out_sb = sb.tile([128, NRT, 2, 384], bf16, tag="out_sb",
                                 name="out_sb")
            for rt in range(NRT):
                for half in range(2):
                    pso = pools["pso"].tile(
                        [128, 384], f32, tag="pso", padded_shape=[128, 512],
                        name="pso")
                    for j in range(3):
                        nc.tensor.matmul(
                            pso[:],
                            d_sb[:, 2 * j:2 * j + 2, rt * 128:(rt + 1) * 128],
                            cas_sb[:, j, :, half * 384:half * 384 + 384],
                            start=(j == 0), stop=(j == 2), perf_mode=DR)
                    e = cfg["f_eng"][rt * 2 + half]
                    if e == "a":
                        nc.scalar.activation(
                            out_sb[:, rt, half, :], pso[:], AF.Copy,
                            scale=INV_N / SCALE)
                    else:
                        eng(e).tensor_scalar_mul(
                            out_sb[:, rt, half, :], pso[:], INV_N / SCALE)
            nc.sync.dma_start(out=out_ap[:, c, :, :, :], in_=out_sb[:])

        for rep in range(repeat):
          pending_final = []
          for c in range(NCHUNK):
            xT8 = mid.tile([128, NT, CHUNK], f8, tag="xT8")
            nc.sync.dma_start(out=xT8[:], in_=xt8_ap[:, c, :, :])

            # emit lagged final stages first: their d is long since ready,
            # giving PE useful work while the xT8 DMA lands
            while len(pending_final) >= max(1, cfg["final_lag"]):
                emit_final(*pending_final.pop(0))

            # ---- fused stage1+layer1: h' = relu(x @ (S1*cas@w1) + S1*b1), fp8
            h_sb = sb.tile([128, NHT, CHUNK], f8, tag="h")
            for t in range(NHT):
                psh = pools["psh"].tile([128, CHUNK], f32, tag="psh")
                for j in range(3):
                    nc.tensor.matmul(
                        psh[:], w1_sb[:, j, :, t, :], xT8[:, 2 * j:2 * j + 2, :],
                        start=(j == 0), stop=(j == 2), perf_mode=DR)
                e = cfg["a_eng"][t]
                if e == "a":
                    nc.scalar.activation(
                        h_sb[:, t, :], psh[:], AF.Relu,
                        bias=b1_sb[:, t:t + 1], scale=1.0)
                else:
                    eng(e).tensor_scalar(
                        h_sb[:, t, :], psh[:], b1_sb[:, t:t + 1], 0.0,
                        ALU.add, ALU.max)

            if cfg["stages"] == "l1":
                continue

            # ---- layer 2 + softshrink difference -> d (fp8, 16x domain)
            d_sb = mid.tile([128, NT, CHUNK], f8, tag="d")
            pend = []

            def ss_tail(tt, vab):
                cab = tmp.tile([128, 2, CHUNK], bf16, tag="cab", name="cab")
                eng(cfg["c_eng"][tt]).tensor_scalar(
                    cab[:], vab[:], -M, M, ALU.max, ALU.min)
                q = tmp.tile([128, CHUNK], bf16, tag="q", name="q")
                eng(cfg["q_eng"][tt]).tensor_sub(q[:], vab[:, 0, :], vab[:, 1, :])
                r = tmp.tile([128, CHUNK], bf16, tag="r", name="r")
                eng(cfg["r_eng"][tt]).tensor_sub(r[:], q[:], cab[:, 0, :])
                eng(cfg["e_eng"][tt]).tensor_add(d_sb[:, tt, :], r[:], cab[:, 1, :])

            for tt in range(NT):
                psab = pools["psab"].tile([128, 2, CHUNK], f32, tag="psab")
                for ri in range(2):
                    for wi, (w0t, cover) in enumerate(_l2_windows(tt)):
                        nc.tensor.matmul(
                            psab[:, ri, :], w2_sb[:, :, ri, tt, wi, :],
                            h_sb[:, w0t:w0t + 2, :],
                            start=(wi == 0), stop=(wi == 1), perf_mode=DR)
                # biased evac: vab_j = psab_j + SCALE*b2_j  (bf16)
                vab = tmp.tile([128, 2, CHUNK], bf16, tag="vab", name="vab")
                for ri in range(2):
                    e = cfg["vb_eng"][tt * 2 + ri]
                    bcol = b2_sb[:, ri, tt:tt + 1]
                    if e == "a":
                        nc.scalar.add(vab[:, ri, :], psab[:, ri, :], bcol)
                    else:
                        nc.vector.tensor_scalar_add(
                            vab[:, ri, :], psab[:, ri, :], bcol)
                if cfg["stages"] == "l2":
                    continue
                pend.append((tt, vab))
                if not cfg["lag"] or len(pend) > 1:
                    ss_tail(*pend.pop(0))
            while pend:
                ss_tail(*pend.pop(0))
            if cfg["stages"] in ("l2", "ss"):
                continue

            pending_final.append((c, d_sb))
          while pending_final:
            emit_final(*pending_final.pop(0))

    if compile:
        nc.compile()
    return nc


def _prep_inputs(x, w1, b1, w2, b2):
    import ml_dtypes
    f8np = ml_dtypes.float8_e4m3

    n = np.arange(C, dtype=np.float64)
    ang = 2.0 * np.pi * n[:, None] * n[None, :] / C
    cas = (np.cos(ang) + np.sin(ang)).astype(np.float32)
    cas8 = np.ascontiguousarray(
        cas.reshape(NT // 2, 2, 128, C).transpose(2, 0, 1, 3)).astype(f8np)

    # fused stage1+layer1 weights, H-channel = (2k+ri)*96 + m
    cas_blocks = cas.reshape(C, NB, BS)
    w1f_full = S1 * np.einsum(
        'akb,rkbm->akrm', cas_blocks.astype(np.float64),
        w1.astype(np.float64)).astype(np.float32)        # [768, NB, 2, 96]
    W1F = w1f_full.reshape(C, 2 * NB * BS)               # [768, 1536]
    w1p = np.ascontiguousarray(
        W1F.reshape(3, 2, 128, NHT, 128).transpose(2, 0, 1, 3, 4)).astype(f8np)

    # layer-2: big block matrices in h'-domain (x SCALE/S1)
    W2A = np.zeros((2 * NB * BS, C), np.float64)   # -> o2r
    W2B = np.zeros((2 * NB * BS, C), np.float64)   # -> o2i
    for k in range(NB):
        hr, hi, c0 = 2 * k * BS, (2 * k + 1) * BS, BS * k
        W2A[hr:hr + BS, c0:c0 + BS] = w2[0][k]
        W2A[hi:hi + BS, c0:c0 + BS] = -w2[1][k]
        W2B[hr:hr + BS, c0:c0 + BS] = w2[1][k]
        W2B[hi:hi + BS, c0:c0 + BS] = w2[0][k]
    W2A *= SCALE / S1
    W2B *= SCALE / S1

    w2p = np.zeros((128, 2, 2, NT, 2, 128), np.float32)
    for tt in range(NT):
        for wi, (w0t, cover) in enumerate(_l2_windows(tt)):
            for s in range(2):
                th = w0t + s
                if th not in cover:
                    continue
                w2p[:, s, 0, tt, wi, :] = W2A[th * 128:(th + 1) * 128,
                                              tt * 128:(tt + 1) * 128]
                w2p[:, s, 1, tt, wi, :] = W2B[th * 128:(th + 1) * 128,
                                              tt * 128:(tt + 1) * 128]
    w2p = w2p.astype(f8np)

    # b2 packed [128, 2, NT] f32 (psab domain, x SCALE)
    b2f = (SCALE * b2.reshape(2, C)).astype(np.float32)
    b2p = np.ascontiguousarray(
        b2f.reshape(2, NT, 128).transpose(2, 0, 1))

    # b1 in h'-domain, packed [128, NHT]
    b1H = (S1 * b1.transpose(1, 0, 2).reshape(2 * NB * BS)).astype(np.float32)
    b1p = np.ascontiguousarray(b1H.reshape(NHT, 128).T)

    shared = {"cas8": cas8, "w1p": w1p, "w2p": w2p, "b2p": b2p, "b1p": b1p}
    maps = []
    for i in range(B):
        xi = np.asarray(x[i], dtype=np.float32)
        # xt8h[p, c, t, r] = x[c*512 + r, t*128 + p]
        xt8h = np.ascontiguousarray(
            xi.T.astype(f8np).reshape(NT, 128, NCHUNK, CHUNK)
            .transpose(1, 2, 0, 3))
        maps.append({"xt8h": xt8h, **shared})
    return maps


class _Runner:
    """Persistent jitted shard_map runner for a compiled Bass module."""

    def __init__(self, nc):
        import jax
        from jax.sharding import Mesh, PartitionSpec, NamedSharding
        from jax.experimental.shard_map import shard_map
        from concourse import mybir
        from concourse.bass2jax import (
            _bass_exec_p, install_neuronx_cc_hook, partition_id_tensor)

        install_neuronx_cc_hook()
        self.jax = jax
        self.nc = nc
        pid_name = nc.partition_id_tensor.name if nc.partition_id_tensor else None
        in_names, out_names, out_avals = [], [], []
        for alloc in nc.m.functions[0].allocations:
            if not isinstance(alloc, mybir.MemoryLocationSet):
                continue
            name = alloc.memorylocations[0].name
            if alloc.kind == "ExternalInput":
                if name != pid_name:
                    in_names.append(name)
            elif alloc.kind == "ExternalOutput":
                out_names.append(name)
                out_avals.append(jax.core.ShapedArray(
                    tuple(alloc.tensor_shape), mybir.dt.np(alloc.dtype)))
        self.in_names, self.out_names, self.out_avals = in_names, out_names, out_avals

        def _body(*args):
            operands = list(args)
            if pid_name is not None:
                operands.append(partition_id_tensor())
            all_names = tuple(in_names) + tuple(out_names) + (
                (pid_name,) if pid_name else ())
            outs = _bass_exec_p.bind(
                *operands,
                out_avals=tuple(out_avals),
                in_names=all_names,
                out_names=tuple(out_names),
                lowering_input_output_aliases=(),
                sim_require_finite=True,
                sim_require_nnan=True,
                nc=nc,
            )
            return tuple(outs)

        devices = jax.devices()[:B]
        self.mesh = Mesh(np.asarray(devices), ("core",))
        nargs = len(in_names) + len(out_names)
        self.sharding = NamedSharding(self.mesh, PartitionSpec("core"))
        self.f = jax.jit(shard_map(
            _body, mesh=self.mesh,
            in_specs=(PartitionSpec("core"),) * nargs,
            out_specs=(PartitionSpec("core"),) * len(out_names),
            check_rep=False,
        ))

    def device_args(self, in_maps):
        concat = [
            np.concatenate([np.asarray(m[n]) for m in in_maps], axis=0)
            for n in self.in_names
        ]
        concat += [
            np.zeros((len(in_maps) * a.shape[0], *a.shape[1:]), a.dtype)
            for a in self.out_avals
        ]
        return [self.jax.device_put(a, self.sharding) for a in concat]

    def run(self, in_maps):
        outs = self.f(*self.device_args(in_maps))
        n = len(in_maps)
        return [
            np.asarray(outs[i]).reshape(n, *self.out_avals[i].shape)
            for i in range(len(self.out_names))
        ]


def get_runner(repeat=1):
    key = ("runner", repeat)
    if key not in _CACHE:
        _CACHE[key] = _Runner(_build(repeat=repeat))
    return _CACHE[key]


def kernel(x, w1, b1, w2, b2):
    x = np.asarray(x, dtype=np.float32)
    w1 = np.asarray(w1, dtype=np.float32)
    b1 = np.asarray(b1, dtype=np.float32)
    w2 = np.asarray(w2, dtype=np.float32)
    b2 = np.asarray(b2, dtype=np.float32)
    runner = get_runner(1)
    in_maps = _prep_inputs(x, w1, b1, w2, b2)
    outh = runner.run(in_maps)[0]      # [B, 128, NCHUNK, NRT, 2, 384]
    # rows r = c*512 + rt*128 + p ; channels = half*384 + u
    delta = outh.astype(np.float32).transpose(0, 2, 3, 1, 4, 5).reshape(B, N, C)
    if CFG["f_dma"]:
        delta *= INV_N / SCALE
    return (x + delta).astype(np.float32)
